# revision 8
# baseline (speedup 1.0000x reference)
"""DeepSets segment-reduce kernel for 8x Trainium2 NeuronCores.

Strategy (all shapes hardcoded for N=500000, C=H=128, O=64, NSEG=2048):
  - Transposed activation layout: features on SBUF partitions, nodes on the
    free axis, so segment reductions are free-axis reduces.
  - Whole-segment sharding: every segment is assigned entirely to one core,
    round-robin by global sorted-width rank.  All 8 cores then share an
    identical compile-time slot/tile geometry (SPMD-safe); per-core padding
    is <1%.  No collective is needed - the host gather is the unshard.
  - Encoder BN is folded into the linear weights (W' = W * g*rsqrt(v+eps),
    b' = (b-m)*g*rsqrt(v+eps) + beta), so each layer is relu(W'x + b').
  - All encoder matmul operands are bf16: the PE streams bf16 moving rows
    at 1 cycle/row vs ~2 for fp32/f32r, and the x DMA halves.  PSUM
    accumulation stays fp32; rel-err vs the fp32 reference is ~2e-3.
  - A large negative pad mask is injected into layer-3 PSUM by a rank-1
    matmul (-BIG x is_pad) over each slot's tail window.  Pad columns then
    fall below zero pre-relu, so they contribute exactly 0 to the post-relu
    segment sums and maxes.
  - Engine balance: relu1/relu2 PSUM->SBUF evacuations run on the Scalar
    (ACT) engine; layer 3 is evacuated by a fused per-slot DVE tensor_scalar
    that adds b3 and simultaneously accumulates the slot's segment-max
    (accum_out); the relu for the sum path runs on the otherwise-idle GpSimd
    (Pool) engine (SBUF bf16 -> bf16); the segment-sum reduce runs on DVE.
    The pre-relu maxes are rectified once in the epilogue.
  - Final projection out = [sum|max|mean] @ Wo'.T + bo' runs per core on its
    own 256 segments; mean is handled by projecting sums through the mean
    block of Wo' and row-scaling by 1/count.
"""

import os
import sys

import numpy as np

if "/opt/trn_rl_repo" not in sys.path:
    sys.path.insert(0, "/opt/trn_rl_repo")

import ml_dtypes

import concourse.bacc as bacc
import concourse.mybir as mybir
import concourse.tile as tile
from concourse import bass_utils

EPS = 1e-5
NSEG = 2048
NCORES = 8
C = 128
H = 128
O = 64
S = NSEG // NCORES  # segment slots per core (256)
MAX_TILE = 512  # PSUM bank / moving-operand limit

BF16 = ml_dtypes.bfloat16

# Per-tile engine assignment patterns for the relu1/relu2 evacuations
# ("a" = Scalar/ACT, "d" = Vector/DVE).  Tuned from trace engine-busy%.
R1_PAT = "a"
R2_PAT = "a"

_compiled_cache = {}


def _fold_bn(W, b, g, be, m, v):
    a = g / np.sqrt(v + EPS)
    Wp = W * a[:, None]
    bp = (b - m) * a + be
    return Wp.astype(np.float32), bp.astype(np.float32)


def _plan_tiles(slot_w):
    """Greedy-pack slots (widths descending) into tiles of <=MAX_TILE cols.

    Returns list of (slot_start, n_slots, padded_width, col_start) and the
    total padded column count.
    """
    tiles = []
    col = 0
    k = 0
    n = len(slot_w)
    while k < n:
        wt = (int(slot_w[k]) + 1) & ~1  # keep matmul widths even
        assert 0 < wt <= MAX_TILE, f"slot width {wt} unsupported"
        d = min(MAX_TILE // wt, n - k)
        tiles.append((k, d, wt, col))
        col += d * wt
        k += d
    return tiles, col


def _build_program(tiles, cols, BIG_DEVICE):
    """Emit the Bass/Tile program shared by all 8 cores."""
    nc = bacc.Bacc(
        "TRN2",
        target_bir_lowering=False,
        debug=False,
        num_devices=NCORES,
    )
    f32 = mybir.dt.float32
    bf16 = mybir.dt.bfloat16

    xT = nc.dram_tensor("xT", [C, cols], bf16, kind="ExternalInput").ap()
    aux = nc.dram_tensor("aux", [1, cols], bf16, kind="ExternalInput").ap()
    w1 = nc.dram_tensor("w1", [C, H], bf16, kind="ExternalInput").ap()
    w2 = nc.dram_tensor("w2", [H, H], bf16, kind="ExternalInput").ap()
    w3 = nc.dram_tensor("w3", [H, H], bf16, kind="ExternalInput").ap()
    b1 = nc.dram_tensor("b1", [H, 1], f32, kind="ExternalInput").ap()
    b2 = nc.dram_tensor("b2", [H, 1], f32, kind="ExternalInput").ap()
    b3 = nc.dram_tensor("b3", [H, 1], f32, kind="ExternalInput").ap()
    nbig = nc.dram_tensor("nbig", [1, H], bf16, kind="ExternalInput").ap()
    wsum = nc.dram_tensor("wsum", [H, O], f32, kind="ExternalInput").ap()
    wmax = nc.dram_tensor("wmax", [H, O], f32, kind="ExternalInput").ap()
    wmean = nc.dram_tensor("wmean", [H, O], f32, kind="ExternalInput").ap()
    bo = nc.dram_tensor("bo", [1, O], f32, kind="ExternalInput").ap()
    # column ch holds the reciprocals for segment chunk ch (128 slots each)
    recip = nc.dram_tensor("recip", [H, S // H], f32, kind="ExternalInput").ap()
    out = nc.dram_tensor("out", [S, O], f32, kind="ExternalOutput").ap()

    relu = mybir.ActivationFunctionType.Relu
    add = mybir.AluOpType.add
    amax = mybir.AluOpType.max

    with tile.TileContext(nc) as tc:
        with (
            tc.tile_pool(name="const", bufs=1) as cpool,
            tc.tile_pool(name="xin", bufs=6) as xpool,
            tc.tile_pool(name="auxin", bufs=6) as apool,
            tc.tile_pool(name="h1", bufs=4) as h1pool,
            tc.tile_pool(name="h2", bufs=4) as h2pool,
            tc.tile_pool(name="t3", bufs=4) as t3pool,
            tc.tile_pool(name="h3", bufs=4) as h3pool,
            tc.tile_pool(name="acc", bufs=1) as accpool,
            tc.tile_pool(name="ps1", bufs=2, space="PSUM") as ps1,
            tc.tile_pool(name="ps2", bufs=2, space="PSUM") as ps2,
            tc.tile_pool(name="ps3", bufs=2, space="PSUM") as ps3,
            tc.tile_pool(name="pso", bufs=1, space="PSUM") as pso,
        ):
            w1s = cpool.tile([C, H], bf16, tag="w1")
            w2s = cpool.tile([H, H], bf16, tag="w2")
            w3s = cpool.tile([H, H], bf16, tag="w3")
            b1s = cpool.tile([H, 1], f32, tag="b1")
            b2s = cpool.tile([H, 1], f32, tag="b2")
            b3s = cpool.tile([H, 1], f32, tag="b3")
            negbig = cpool.tile([1, H], bf16, tag="negbig")
            wsums = cpool.tile([H, O], f32, tag="wsum")
            wmaxs = cpool.tile([H, O], f32, tag="wmax")
            wmeans = cpool.tile([H, O], f32, tag="wmean")
            bos = cpool.tile([1, O], f32, tag="bo")
            recs = cpool.tile([H, S // H], f32, tag="recip")
            ones = cpool.tile([1, H], f32, tag="ones")

            nc.sync.dma_start(w1s[:], w1)
            nc.sync.dma_start(w2s[:], w2)
            nc.sync.dma_start(w3s[:], w3)
            nc.sync.dma_start(b1s[:], b1)
            nc.sync.dma_start(b2s[:], b2)
            nc.sync.dma_start(b3s[:], b3)
            nc.sync.dma_start(wsums[:], wsum)
            nc.sync.dma_start(wmaxs[:], wmax)
            nc.sync.dma_start(wmeans[:], wmean)
            nc.sync.dma_start(bos[:], bo)
            nc.sync.dma_start(recs[:], recip)
            nc.vector.memset(ones[:], 1.0)
            nc.sync.dma_start(negbig[:], nbig)

            # Persistent per-slot partials (post-relu sums and maxes).
            sumP = accpool.tile([H, S], f32, tag="sumP")
            maxP = accpool.tile([H, S], f32, tag="maxP")

            def relu_evac(eng, dst, src, bias):
                if eng == "a":
                    nc.scalar.activation(dst, src, relu, bias=bias)
                else:
                    nc.vector.tensor_scalar(
                        dst, src, bias, 0.0, op0=add, op1=amax
                    )

            for ti, (k0, d, wt, col0, tailw) in enumerate(tiles):
                tcols = d * wt
                e1 = R1_PAT[ti % len(R1_PAT)]
                e2 = R2_PAT[ti % len(R2_PAT)]

                xt = xpool.tile([C, MAX_TILE], bf16, tag="xt")
                at = apool.tile([1, MAX_TILE], bf16, tag="at")
                nc.sync.dma_start(xt[:, :tcols], xT[:, col0 : col0 + tcols])
                if tailw > 0:
                    nc.sync.dma_start(at[:, :tcols], aux[:, col0 : col0 + tcols])

                p1 = ps1.tile([H, MAX_TILE], f32, tag="p1")
                nc.tensor.matmul(p1[:, :tcols], w1s[:], xt[:, :tcols])
                h1 = h1pool.tile([H, MAX_TILE], bf16, tag="h1")
                relu_evac(e1, h1[:, :tcols], p1[:, :tcols], b1s[:])

                p2 = ps2.tile([H, MAX_TILE], f32, tag="p2")
                nc.tensor.matmul(p2[:, :tcols], w2s[:], h1[:, :tcols])
                h2 = h2pool.tile([H, MAX_TILE], bf16, tag="h2")
                relu_evac(e2, h2[:, :tcols], p2[:, :tcols], b2s[:])

                p3 = ps3.tile([H, MAX_TILE], f32, tag="p3")
                p3v = p3[:, :tcols].rearrange("p (d w) -> p d w", d=d)
                if tailw > 0:
                    nc.tensor.matmul(
                        p3[:, :tcols], w3s[:], h2[:, :tcols], start=True, stop=False
                    )
                    # -BIG into the per-slot tail windows (the only columns
                    # that can be pads on any core); real columns add 0.
                    atv = at[:, :tcols].rearrange("p (d w) -> p d w", d=d)
                    nc.tensor.matmul(
                        p3v[:, :, wt - tailw : wt],
                        negbig[:],
                        atv[:, :, wt - tailw : wt],
                        start=False,
                        stop=True,
                    )
                else:
                    nc.tensor.matmul(
                        p3[:, :tcols], w3s[:], h2[:, :tcols], start=True, stop=True
                    )

                # Fused per-slot bias-add + segment-max on DVE: t3 = p3 + b3
                # (pre-relu, bf16) and maxP[:, k] = max over the slot window.
                t3 = t3pool.tile([H, MAX_TILE], bf16, tag="t3")
                for j in range(d):
                    wl = slice(j * wt, (j + 1) * wt)
                    nc.vector.tensor_scalar(
                        t3[:, wl],
                        p3[:, wl],
                        b3s[:],
                        None,
                        op0=add,
                        op1=amax,
                        accum_out=maxP[:, k0 + j : k0 + j + 1],
                    )
                # relu on the otherwise-idle GpSimd engine (SBUF bf16 only).
                h3 = h3pool.tile([H, MAX_TILE], bf16, tag="h3")
                nc.gpsimd.tensor_scalar_max(h3[:, :tcols], t3[:, :tcols], 0.0)
                h3v = h3[:, :tcols].rearrange("p (d w) -> p d w", d=d)
                nc.vector.reduce_sum(
                    sumP[:, k0 : k0 + d], h3v, axis=mybir.AxisListType.X
                )

            # ---- epilogue: out[k, :] = sum_k @ Wsum + relu(max_k) @ Wmax
            #                + (sum_k * recip_k) @ Wmean + bo ----
            maxR = accpool.tile([H, S], f32, tag="maxR")
            nc.scalar.activation(maxR[:], maxP[:], relu, bias=0.0)

            for ch in range(S // H):  # 2 chunks of 128 segments
                sl = slice(ch * H, (ch + 1) * H)
                po = pso.tile([H, O], f32, tag="po")
                nc.tensor.matmul(po[:], sumP[:, sl], wsums[:], start=True, stop=False)
                nc.tensor.matmul(po[:], maxR[:, sl], wmaxs[:], start=False, stop=False)
                nc.tensor.matmul(po[:], ones[:], bos[:], start=False, stop=True)

                pm = pso.tile([H, O], f32, tag="pm")
                nc.tensor.matmul(pm[:], sumP[:, sl], wmeans[:], start=True, stop=True)

                om = h1pool.tile([H, O], f32, tag="om")
                nc.vector.tensor_scalar_mul(om[:], pm[:], recs[:, ch : ch + 1])
                ot = h2pool.tile([H, O], f32, tag="ot")
                nc.vector.tensor_tensor(ot[:], po[:], om[:], op=mybir.AluOpType.add)
                nc.sync.dma_start(out[sl, :], ot[:])

    nc.compile()
    return nc


def kernel(**inputs):
    x = np.ascontiguousarray(np.asarray(inputs["x"], dtype=np.float32))
    batch = np.asarray(inputs["batch"]).astype(np.int64)

    # ---- fold BN into the linears ----
    W1p, b1p = _fold_bn(
        np.asarray(inputs["W1"]), np.asarray(inputs["b1"]),
        np.asarray(inputs["g1"]), np.asarray(inputs["be1"]),
        np.asarray(inputs["m1"]), np.asarray(inputs["v1"]),
    )
    W2p, b2p = _fold_bn(
        np.asarray(inputs["W2"]), np.asarray(inputs["b2"]),
        np.asarray(inputs["g2"]), np.asarray(inputs["be2"]),
        np.asarray(inputs["m2"]), np.asarray(inputs["v2"]),
    )
    W3p, b3p = _fold_bn(
        np.asarray(inputs["W3"]), np.asarray(inputs["b3"]),
        np.asarray(inputs["g3"]), np.asarray(inputs["be3"]),
        np.asarray(inputs["m3"]), np.asarray(inputs["v3"]),
    )
    Wop, bop = _fold_bn(
        np.asarray(inputs["Wo"]), np.asarray(inputs["bo"]),
        np.asarray(inputs["go"]), np.asarray(inputs["beo"]),
        np.asarray(inputs["mo"]), np.asarray(inputs["vo"]),
    )

    # Pad columns are zero in x, so h2_pad is a known constant; BIG pushes the
    # padded layer-3 pre-activation strictly below zero.  Round BIG to a
    # bf16-exact value.
    h1_pad = np.maximum(b1p, 0.0)
    h2_pad = np.maximum(W2p @ h1_pad + b2p, 0.0)
    v3 = W3p @ h2_pad + b3p
    BIG = float(np.float32(BF16(max(0.0, float(v3.max())) + 1024.0)))

    # ---- whole-segment sharding by sorted-width round-robin rank ----
    counts = np.bincount(batch, minlength=NSEG).astype(np.int64)
    assert np.all(batch[:-1] <= batch[1:]), "batch must be sorted"
    order = np.argsort(-counts, kind="stable")  # segment ids, width desc
    slot_w = np.maximum(counts[order[::NCORES][:S]], 1)  # width of rank 8k
    tiles0, cols = _plan_tiles(slot_w)

    # per-tile tail-window width: the trailing columns of each slot that can
    # be padding on ANY core (only these need the -BIG mask matmul)
    wmat = counts[order[: S * NCORES]].reshape(S, NCORES)  # slot x core widths
    tiles = []
    for k0, d, wt, col0 in tiles0:
        minw = int(wmat[k0 : k0 + d].min())
        tailw = min(wt, (wt - minw + 1) & ~1)
        tiles.append((k0, d, wt, col0, tailw))

    key = (cols, float(BIG), tuple(slot_w.tolist()),
           tuple(t[4] for t in tiles))
    if key not in _compiled_cache:
        _compiled_cache[key] = _build_program(tiles, cols, BIG)
    nc = _compiled_cache[key]

    # column start of each slot
    slot_col = np.zeros(S, dtype=np.int64)
    for k0, d, wt, col0, tailw in tiles:
        for j in range(d):
            slot_col[k0 + j] = col0 + j * wt

    starts = np.searchsorted(batch, np.arange(NSEG), side="left")
    ends = np.searchsorted(batch, np.arange(NSEG), side="right")

    in_maps = []
    for c in range(NCORES):
        segs = order[np.arange(S) * NCORES + c]  # this core's segment ids
        src = np.full(cols, -1, dtype=np.int64)
        for k in range(S):
            s = segs[k]
            cnt = int(counts[s])
            if cnt:
                src[slot_col[k] : slot_col[k] + cnt] = np.arange(
                    starts[s], ends[s]
                )
        real = src >= 0
        xTc = np.zeros((C, cols), dtype=BF16)
        xTc[:, real] = x[src[real]].astype(BF16).T
        auxc = np.zeros((1, cols), dtype=BF16)
        auxc[0, ~real] = 1.0
        recipc = (1.0 / np.maximum(counts[segs], 1.0)).astype(np.float32)
        in_maps.append(
            dict(
                xT=xTc,
                aux=auxc,
                w1=np.ascontiguousarray(W1p.T.astype(BF16)),
                w2=np.ascontiguousarray(W2p.T.astype(BF16)),
                w3=np.ascontiguousarray(W3p.T.astype(BF16)),
                b1=np.ascontiguousarray(b1p[:, None]),
                b2=np.ascontiguousarray(b2p[:, None]),
                b3=np.ascontiguousarray(b3p[:, None]),
                nbig=np.full((1, H), -BIG, BF16),
                wsum=np.ascontiguousarray(Wop[:, 0:H].T),
                wmax=np.ascontiguousarray(Wop[:, H : 2 * H].T),
                wmean=np.ascontiguousarray(Wop[:, 2 * H : 3 * H].T),
                bo=np.ascontiguousarray(bop[None, :]),
                recip=np.ascontiguousarray(recipc.reshape(S // H, H).T),
            )
        )

    ncores_run = int(os.environ.get("KERNEL_NCORES", str(NCORES)))
    res = bass_utils.run_bass_kernel_spmd(
        nc,
        in_maps[:ncores_run],
        core_ids=list(range(ncores_run)),
        trace=bool(int(os.environ.get("KERNEL_TRACE", "0"))),
        tmpdir=os.environ.get("KERNEL_TRACE_DIR") or None,
    )
    kernel.last_results = res

    out_full = np.zeros((NSEG, O), dtype=np.float32)
    ranks = np.arange(S)
    for c in range(ncores_run):
        out_full[order[ranks * NCORES + c]] = res.results[c]["out"]
    return out_full


# revision 14
# speedup vs baseline: 3.5248x; 3.5248x over previous
"""DeepSets segment-reduce kernel for 8x Trainium2 NeuronCores.

Strategy (all shapes hardcoded for N=500000, C=H=128, O=64, NSEG=2048):
  - Transposed activation layout: features on SBUF partitions, nodes on the
    free axis, so segment reductions are free-axis reduces.
  - Whole-segment sharding: every segment is assigned entirely to one core,
    round-robin by global sorted-width rank.  All 8 cores then share an
    identical compile-time slot/tile geometry (SPMD-safe); per-core padding
    is <1%.  No collective is needed - the host gather is the unshard.
  - Encoder BN is folded into the linear weights (W' = W * g*rsqrt(v+eps),
    b' = (b-m)*g*rsqrt(v+eps) + beta), so each layer is relu(W'x + b').
  - All encoder matmul operands are bf16: the PE streams bf16 moving rows
    at 1 cycle/row vs ~2 for fp32/f32r, and the x DMA halves.  PSUM
    accumulation stays fp32; rel-err vs the fp32 reference is ~2e-3.
  - A large negative pad mask is injected into layer-3 PSUM by a rank-1
    matmul (-BIG x is_pad) over each slot's tail window.  Pad columns then
    fall below zero pre-relu, so they contribute exactly 0 to the post-relu
    segment sums and maxes.
  - Engine balance: relu1/relu3 PSUM->SBUF evacuations run on the Scalar
    (ACT) engine and relu2 on the Vector (DVE) engine.  The segment sum and
    max reduces are per-slot DVE tensor_scalar ops with accum_out reading
    the bf16 h3 from SBUF: unlike tensor_reduce (1x only), tensor_scalar
    runs in the DVE 4x perf mode (bf16, SBUF, packed), so both reduces cost
    ~1/4 of a regular pass.  h3 is post-relu with bias applied and pads
    exactly zero, so maxP needs no epilogue fixup.
  - Final projection out = [sum|max|mean] @ Wo'.T + bo' runs per core on its
    own 256 segments; mean is handled by projecting sums through the mean
    block of Wo' and row-scaling by 1/count.
"""

import os
import sys

import numpy as np

if "/opt/trn_rl_repo" not in sys.path:
    sys.path.insert(0, "/opt/trn_rl_repo")

import ml_dtypes

import concourse.bacc as bacc
import concourse.mybir as mybir
import concourse.tile as tile
from concourse import bass_utils

EPS = 1e-5
NSEG = 2048
NCORES = 8
C = 128
H = 128
O = 64
S = NSEG // NCORES  # segment slots per core (256)
MAX_TILE = 512  # PSUM bank / moving-operand limit

BF16 = ml_dtypes.bfloat16

# Per-tile engine assignment patterns for the relu1/2/3 evacuations
# ("a" = Scalar/ACT, "d" = Vector/DVE).  Tuned from trace engine-busy%.
R1_PAT = "a"
R2_PAT = "d"
R3_PAT = "a"

_compiled_cache = {}


def _fold_bn(W, b, g, be, m, v):
    a = g / np.sqrt(v + EPS)
    Wp = W * a[:, None]
    bp = (b - m) * a + be
    return Wp.astype(np.float32), bp.astype(np.float32)


def _plan_tiles(slot_w):
    """Greedy-pack slots (widths descending) into tiles of <=MAX_TILE cols.

    Returns list of (slot_start, n_slots, padded_width, col_start) and the
    total padded column count.
    """
    tiles = []
    col = 0
    k = 0
    n = len(slot_w)
    while k < n:
        wt = (int(slot_w[k]) + 1) & ~1  # keep matmul widths even
        assert 0 < wt <= MAX_TILE, f"slot width {wt} unsupported"
        d = min(MAX_TILE // wt, n - k)
        tiles.append((k, d, wt, col))
        col += d * wt
        k += d
    return tiles, col


def _build_program(tiles, cols, BIG_DEVICE):
    """Emit the Bass/Tile program shared by all 8 cores."""
    nc = bacc.Bacc(
        "TRN2",
        target_bir_lowering=False,
        debug=False,
        num_devices=NCORES,
    )
    f32 = mybir.dt.float32
    bf16 = mybir.dt.bfloat16

    xT = nc.dram_tensor("xT", [C, cols], bf16, kind="ExternalInput").ap()
    aux = nc.dram_tensor("aux", [1, cols], bf16, kind="ExternalInput").ap()
    w1 = nc.dram_tensor("w1", [C, H], bf16, kind="ExternalInput").ap()
    w2 = nc.dram_tensor("w2", [H, H], bf16, kind="ExternalInput").ap()
    w3 = nc.dram_tensor("w3", [H, H], bf16, kind="ExternalInput").ap()
    b1 = nc.dram_tensor("b1", [H, 1], f32, kind="ExternalInput").ap()
    b2 = nc.dram_tensor("b2", [H, 1], f32, kind="ExternalInput").ap()
    b3 = nc.dram_tensor("b3", [H, 1], f32, kind="ExternalInput").ap()
    nbig = nc.dram_tensor("nbig", [1, H], bf16, kind="ExternalInput").ap()
    wsum = nc.dram_tensor("wsum", [H, O], f32, kind="ExternalInput").ap()
    wmax = nc.dram_tensor("wmax", [H, O], f32, kind="ExternalInput").ap()
    wmean = nc.dram_tensor("wmean", [H, O], f32, kind="ExternalInput").ap()
    bo = nc.dram_tensor("bo", [1, O], f32, kind="ExternalInput").ap()
    # column ch holds the reciprocals for segment chunk ch (128 slots each)
    recip = nc.dram_tensor("recip", [H, S // H], f32, kind="ExternalInput").ap()
    out = nc.dram_tensor("out", [S, O], f32, kind="ExternalOutput").ap()

    relu = mybir.ActivationFunctionType.Relu
    add = mybir.AluOpType.add
    amax = mybir.AluOpType.max

    with tile.TileContext(nc) as tc:
        with (
            tc.tile_pool(name="const", bufs=1) as cpool,
            tc.tile_pool(name="xin", bufs=6) as xpool,
            tc.tile_pool(name="auxin", bufs=6) as apool,
            tc.tile_pool(name="h1", bufs=4) as h1pool,
            tc.tile_pool(name="h2", bufs=4) as h2pool,
            tc.tile_pool(name="h3", bufs=4) as h3pool,
            tc.tile_pool(name="scrS", bufs=2) as scrSpool,
            tc.tile_pool(name="scrM", bufs=2) as scrMpool,
            tc.tile_pool(name="acc", bufs=1) as accpool,
            tc.tile_pool(name="ps1", bufs=2, space="PSUM") as ps1,
            tc.tile_pool(name="ps2", bufs=2, space="PSUM") as ps2,
            tc.tile_pool(name="ps3", bufs=2, space="PSUM") as ps3,
            tc.tile_pool(name="pso", bufs=1, space="PSUM") as pso,
        ):
            w1s = cpool.tile([C, H], bf16, tag="w1")
            w2s = cpool.tile([H, H], bf16, tag="w2")
            w3s = cpool.tile([H, H], bf16, tag="w3")
            b1s = cpool.tile([H, 1], f32, tag="b1")
            b2s = cpool.tile([H, 1], f32, tag="b2")
            b3s = cpool.tile([H, 1], f32, tag="b3")
            negbig = cpool.tile([1, H], bf16, tag="negbig")
            wsums = cpool.tile([H, O], f32, tag="wsum")
            wmaxs = cpool.tile([H, O], f32, tag="wmax")
            wmeans = cpool.tile([H, O], f32, tag="wmean")
            bos = cpool.tile([1, O], f32, tag="bo")
            recs = cpool.tile([H, S // H], f32, tag="recip")
            ones = cpool.tile([1, H], f32, tag="ones")

            nc.sync.dma_start(w1s[:], w1)
            nc.sync.dma_start(w2s[:], w2)
            nc.sync.dma_start(w3s[:], w3)
            nc.sync.dma_start(b1s[:], b1)
            nc.sync.dma_start(b2s[:], b2)
            nc.sync.dma_start(b3s[:], b3)
            nc.sync.dma_start(wsums[:], wsum)
            nc.sync.dma_start(wmaxs[:], wmax)
            nc.sync.dma_start(wmeans[:], wmean)
            nc.sync.dma_start(bos[:], bo)
            nc.sync.dma_start(recs[:], recip)
            nc.vector.memset(ones[:], 1.0)
            nc.sync.dma_start(negbig[:], nbig)

            # Persistent per-slot partials (post-relu sums and maxes).
            sumP = accpool.tile([H, S], f32, tag="sumP")
            maxP = accpool.tile([H, S], f32, tag="maxP")

            def relu_evac(eng, dst, src, bias):
                if eng == "a":
                    nc.scalar.activation(dst, src, relu, bias=bias)
                else:
                    nc.vector.tensor_scalar(
                        dst, src, bias, 0.0, op0=add, op1=amax
                    )

            for ti, (k0, d, wt, col0, tailw) in enumerate(tiles):
                tcols = d * wt
                e1 = R1_PAT[ti % len(R1_PAT)]
                e2 = R2_PAT[ti % len(R2_PAT)]
                e3 = R3_PAT[ti % len(R3_PAT)]

                xt = xpool.tile([C, MAX_TILE], bf16, tag="xt")
                at = apool.tile([1, MAX_TILE], bf16, tag="at")
                nc.sync.dma_start(xt[:, :tcols], xT[:, col0 : col0 + tcols])
                if tailw > 0:
                    nc.sync.dma_start(at[:, :tcols], aux[:, col0 : col0 + tcols])

                p1 = ps1.tile([H, MAX_TILE], f32, tag="p1")
                nc.tensor.matmul(p1[:, :tcols], w1s[:], xt[:, :tcols])
                h1 = h1pool.tile([H, MAX_TILE], bf16, tag="h1")
                relu_evac(e1, h1[:, :tcols], p1[:, :tcols], b1s[:])

                p2 = ps2.tile([H, MAX_TILE], f32, tag="p2")
                nc.tensor.matmul(p2[:, :tcols], w2s[:], h1[:, :tcols])
                h2 = h2pool.tile([H, MAX_TILE], bf16, tag="h2")
                relu_evac(e2, h2[:, :tcols], p2[:, :tcols], b2s[:])

                p3 = ps3.tile([H, MAX_TILE], f32, tag="p3")
                p3v = p3[:, :tcols].rearrange("p (d w) -> p d w", d=d)
                if tailw > 0:
                    nc.tensor.matmul(
                        p3[:, :tcols], w3s[:], h2[:, :tcols], start=True, stop=False
                    )
                    # -BIG into the per-slot tail windows (the only columns
                    # that can be pads on any core); real columns add 0.
                    atv = at[:, :tcols].rearrange("p (d w) -> p d w", d=d)
                    nc.tensor.matmul(
                        p3v[:, :, wt - tailw : wt],
                        negbig[:],
                        atv[:, :, wt - tailw : wt],
                        start=False,
                        stop=True,
                    )
                else:
                    nc.tensor.matmul(
                        p3[:, :tcols], w3s[:], h2[:, :tcols], start=True, stop=True
                    )

                h3 = h3pool.tile([H, MAX_TILE], bf16, tag="h3")
                relu_evac(e3, h3[:, :tcols], p3[:, :tcols], b3s[:])

                # Per-slot segment sum + max as DVE tensor_scalar ops with
                # accum_out.  op0 = max(h3, 0.0) is the identity on the
                # post-relu h3 but keeps the op in tensor_scalar form, which
                # (unlike tensor_reduce) runs in the 4x DVE perf mode for
                # packed bf16 SBUF operands.  op1 selects the reduction.
                scrS = scrSpool.tile([H, MAX_TILE], bf16, tag="scrS")
                scrM = scrMpool.tile([H, MAX_TILE], bf16, tag="scrM")
                for j in range(d):
                    wl = slice(j * wt, (j + 1) * wt)
                    nc.vector.tensor_scalar(
                        scrS[:, wl], h3[:, wl], 0.0, None,
                        op0=amax, op1=add,
                        accum_out=sumP[:, k0 + j : k0 + j + 1],
                    )
                    nc.vector.tensor_scalar(
                        scrM[:, wl], h3[:, wl], 0.0, None,
                        op0=amax, op1=amax,
                        accum_out=maxP[:, k0 + j : k0 + j + 1],
                    )

            # ---- epilogue: out[k, :] = sum_k @ Wsum + max_k @ Wmax
            #                + (sum_k * recip_k) @ Wmean + bo ----
            for ch in range(S // H):  # 2 chunks of 128 segments
                sl = slice(ch * H, (ch + 1) * H)
                po = pso.tile([H, O], f32, tag="po")
                nc.tensor.matmul(po[:], sumP[:, sl], wsums[:], start=True, stop=False)
                nc.tensor.matmul(po[:], maxP[:, sl], wmaxs[:], start=False, stop=False)
                nc.tensor.matmul(po[:], ones[:], bos[:], start=False, stop=True)

                pm = pso.tile([H, O], f32, tag="pm")
                nc.tensor.matmul(pm[:], sumP[:, sl], wmeans[:], start=True, stop=True)

                om = h1pool.tile([H, O], f32, tag="om")
                nc.vector.tensor_scalar_mul(om[:], pm[:], recs[:, ch : ch + 1])
                ot = h2pool.tile([H, O], f32, tag="ot")
                nc.vector.tensor_tensor(ot[:], po[:], om[:], op=mybir.AluOpType.add)
                nc.sync.dma_start(out[sl, :], ot[:])

    nc.compile()
    return nc


def kernel(**inputs):
    x = np.ascontiguousarray(np.asarray(inputs["x"], dtype=np.float32))
    batch = np.asarray(inputs["batch"]).astype(np.int64)

    # ---- fold BN into the linears ----
    W1p, b1p = _fold_bn(
        np.asarray(inputs["W1"]), np.asarray(inputs["b1"]),
        np.asarray(inputs["g1"]), np.asarray(inputs["be1"]),
        np.asarray(inputs["m1"]), np.asarray(inputs["v1"]),
    )
    W2p, b2p = _fold_bn(
        np.asarray(inputs["W2"]), np.asarray(inputs["b2"]),
        np.asarray(inputs["g2"]), np.asarray(inputs["be2"]),
        np.asarray(inputs["m2"]), np.asarray(inputs["v2"]),
    )
    W3p, b3p = _fold_bn(
        np.asarray(inputs["W3"]), np.asarray(inputs["b3"]),
        np.asarray(inputs["g3"]), np.asarray(inputs["be3"]),
        np.asarray(inputs["m3"]), np.asarray(inputs["v3"]),
    )
    Wop, bop = _fold_bn(
        np.asarray(inputs["Wo"]), np.asarray(inputs["bo"]),
        np.asarray(inputs["go"]), np.asarray(inputs["beo"]),
        np.asarray(inputs["mo"]), np.asarray(inputs["vo"]),
    )

    # Pad columns are zero in x, so h2_pad is a known constant; BIG pushes the
    # padded layer-3 pre-activation strictly below zero.  Round BIG to a
    # bf16-exact value.
    h1_pad = np.maximum(b1p, 0.0)
    h2_pad = np.maximum(W2p @ h1_pad + b2p, 0.0)
    v3 = W3p @ h2_pad + b3p
    BIG = float(np.float32(BF16(max(0.0, float(v3.max())) + 1024.0)))

    # ---- whole-segment sharding by sorted-width round-robin rank ----
    counts = np.bincount(batch, minlength=NSEG).astype(np.int64)
    assert np.all(batch[:-1] <= batch[1:]), "batch must be sorted"
    order = np.argsort(-counts, kind="stable")  # segment ids, width desc
    slot_w = np.maximum(counts[order[::NCORES][:S]], 1)  # width of rank 8k
    tiles0, cols = _plan_tiles(slot_w)

    # per-tile tail-window width: the trailing columns of each slot that can
    # be padding on ANY core (only these need the -BIG mask matmul)
    wmat = counts[order[: S * NCORES]].reshape(S, NCORES)  # slot x core widths
    tiles = []
    for k0, d, wt, col0 in tiles0:
        minw = int(wmat[k0 : k0 + d].min())
        tailw = min(wt, (wt - minw + 1) & ~1)
        tiles.append((k0, d, wt, col0, tailw))

    key = (cols, float(BIG), tuple(slot_w.tolist()),
           tuple(t[4] for t in tiles))
    if key not in _compiled_cache:
        _compiled_cache[key] = _build_program(tiles, cols, BIG)
    nc = _compiled_cache[key]

    # column start of each slot
    slot_col = np.zeros(S, dtype=np.int64)
    for k0, d, wt, col0, tailw in tiles:
        for j in range(d):
            slot_col[k0 + j] = col0 + j * wt

    starts = np.searchsorted(batch, np.arange(NSEG), side="left")
    ends = np.searchsorted(batch, np.arange(NSEG), side="right")

    in_maps = []
    for c in range(NCORES):
        segs = order[np.arange(S) * NCORES + c]  # this core's segment ids
        src = np.full(cols, -1, dtype=np.int64)
        for k in range(S):
            s = segs[k]
            cnt = int(counts[s])
            if cnt:
                src[slot_col[k] : slot_col[k] + cnt] = np.arange(
                    starts[s], ends[s]
                )
        real = src >= 0
        xTc = np.zeros((C, cols), dtype=BF16)
        xTc[:, real] = x[src[real]].astype(BF16).T
        auxc = np.zeros((1, cols), dtype=BF16)
        auxc[0, ~real] = 1.0
        recipc = (1.0 / np.maximum(counts[segs], 1.0)).astype(np.float32)
        in_maps.append(
            dict(
                xT=xTc,
                aux=auxc,
                w1=np.ascontiguousarray(W1p.T.astype(BF16)),
                w2=np.ascontiguousarray(W2p.T.astype(BF16)),
                w3=np.ascontiguousarray(W3p.T.astype(BF16)),
                b1=np.ascontiguousarray(b1p[:, None]),
                b2=np.ascontiguousarray(b2p[:, None]),
                b3=np.ascontiguousarray(b3p[:, None]),
                nbig=np.full((1, H), -BIG, BF16),
                wsum=np.ascontiguousarray(Wop[:, 0:H].T),
                wmax=np.ascontiguousarray(Wop[:, H : 2 * H].T),
                wmean=np.ascontiguousarray(Wop[:, 2 * H : 3 * H].T),
                bo=np.ascontiguousarray(bop[None, :]),
                recip=np.ascontiguousarray(recipc.reshape(S // H, H).T),
            )
        )

    ncores_run = int(os.environ.get("KERNEL_NCORES", str(NCORES)))
    res = bass_utils.run_bass_kernel_spmd(
        nc,
        in_maps[:ncores_run],
        core_ids=list(range(ncores_run)),
        trace=bool(int(os.environ.get("KERNEL_TRACE", "0"))),
        tmpdir=os.environ.get("KERNEL_TRACE_DIR") or None,
    )
    kernel.last_results = res

    out_full = np.zeros((NSEG, O), dtype=np.float32)
    ranks = np.arange(S)
    for c in range(ncores_run):
        out_full[order[ranks * NCORES + c]] = res.results[c]["out"]
    return out_full


# revision 18
# speedup vs baseline: 3.7678x; 1.0689x over previous
"""DeepSets segment-reduce kernel for 8x Trainium2 NeuronCores.

Strategy (all shapes hardcoded for N=500000, C=H=128, O=64, NSEG=2048):
  - Transposed activation layout: features on SBUF partitions, nodes on the
    free axis, so segment reductions are free-axis reduces.
  - Whole-segment sharding: every segment is assigned entirely to one core,
    round-robin by global sorted-width rank.  All 8 cores then share an
    identical compile-time slot/tile geometry (SPMD-safe); per-core padding
    is <1%.  No collective is needed - the host gather is the unshard.
  - Encoder BN is folded into the linear weights (W' = W * g*rsqrt(v+eps),
    b' = (b-m)*g*rsqrt(v+eps) + beta), so each layer is relu(W'x + b').
  - All encoder matmul operands are bf16: the PE streams bf16 moving rows
    at 1 cycle/row vs ~2 for fp32/f32r, and the x DMA halves.  PSUM
    accumulation stays fp32; rel-err vs the fp32 reference is ~2e-3.
  - A large negative pad mask is injected into layer-3 PSUM by a rank-1
    matmul (-BIG x is_pad) over each slot's tail window.  Pad columns then
    fall below zero pre-relu, so they contribute exactly 0 to the post-relu
    segment sums and maxes.
  - Engine balance: layer 3 is evacuated by per-slot Scalar-engine
    activation ops whose accum_out accumulates the slot's post-relu segment
    sum for free; relu2 runs on the Vector engine and relu1 is split between
    the two by a per-tile pattern.  The segment max is a whole-tile 3D
    reduce_max on DVE over the bf16 h3 (post-relu, bias applied, pads zero -
    so maxP needs no epilogue fixup).
  - Tiles are processed in pairs with matmuls grouped per layer, so each
    PE weight load (LdWeights) covers two 512-column matmuls.
  - Final projection out = [sum|max|mean] @ Wo'.T + bo' runs per core on its
    own 256 segments; mean is handled by projecting sums through the mean
    block of Wo' and row-scaling by 1/count.
"""

import os
import sys

import numpy as np

if "/opt/trn_rl_repo" not in sys.path:
    sys.path.insert(0, "/opt/trn_rl_repo")

import ml_dtypes

import concourse.bacc as bacc
import concourse.mybir as mybir
import concourse.tile as tile
from concourse import bass_utils

EPS = 1e-5
NSEG = 2048
NCORES = 8
C = 128
H = 128
O = 64
S = NSEG // NCORES  # segment slots per core (256)
MAX_TILE = 512  # PSUM bank / moving-operand limit

BF16 = ml_dtypes.bfloat16

# Per-tile engine assignment patterns for the relu1/relu2 evacuations
# ("a" = Scalar/ACT, "d" = Vector/DVE).  Tuned from trace engine-busy%.
R1_PAT = "aad"
R2_PAT = "d"

_compiled_cache = {}


def _fold_bn(W, b, g, be, m, v):
    a = g / np.sqrt(v + EPS)
    Wp = W * a[:, None]
    bp = (b - m) * a + be
    return Wp.astype(np.float32), bp.astype(np.float32)


def _plan_tiles(slot_w):
    """Greedy-pack slots (widths descending) into tiles of <=MAX_TILE cols.

    Returns list of (slot_start, n_slots, padded_width, col_start) and the
    total padded column count.
    """
    tiles = []
    col = 0
    k = 0
    n = len(slot_w)
    while k < n:
        wt = (int(slot_w[k]) + 1) & ~1  # keep matmul widths even
        assert 0 < wt <= MAX_TILE, f"slot width {wt} unsupported"
        d = min(MAX_TILE // wt, n - k)
        tiles.append((k, d, wt, col))
        col += d * wt
        k += d
    return tiles, col


def _build_program(tiles, cols, BIG_DEVICE):
    """Emit the Bass/Tile program shared by all 8 cores."""
    nc = bacc.Bacc(
        "TRN2",
        target_bir_lowering=False,
        debug=False,
        num_devices=NCORES,
    )
    f32 = mybir.dt.float32
    bf16 = mybir.dt.bfloat16

    xT = nc.dram_tensor("xT", [C, cols], bf16, kind="ExternalInput").ap()
    aux = nc.dram_tensor("aux", [1, cols], bf16, kind="ExternalInput").ap()
    w1 = nc.dram_tensor("w1", [C, H], bf16, kind="ExternalInput").ap()
    w2 = nc.dram_tensor("w2", [H, H], bf16, kind="ExternalInput").ap()
    w3 = nc.dram_tensor("w3", [H, H], bf16, kind="ExternalInput").ap()
    b1 = nc.dram_tensor("b1", [H, 1], f32, kind="ExternalInput").ap()
    b2 = nc.dram_tensor("b2", [H, 1], f32, kind="ExternalInput").ap()
    b3 = nc.dram_tensor("b3", [H, 1], f32, kind="ExternalInput").ap()
    nbig = nc.dram_tensor("nbig", [1, H], bf16, kind="ExternalInput").ap()
    wsum = nc.dram_tensor("wsum", [H, O], f32, kind="ExternalInput").ap()
    wmax = nc.dram_tensor("wmax", [H, O], f32, kind="ExternalInput").ap()
    wmean = nc.dram_tensor("wmean", [H, O], f32, kind="ExternalInput").ap()
    bo = nc.dram_tensor("bo", [1, O], f32, kind="ExternalInput").ap()
    # column ch holds the reciprocals for segment chunk ch (128 slots each)
    recip = nc.dram_tensor("recip", [H, S // H], f32, kind="ExternalInput").ap()
    out = nc.dram_tensor("out", [S, O], f32, kind="ExternalOutput").ap()

    relu = mybir.ActivationFunctionType.Relu
    add = mybir.AluOpType.add
    amax = mybir.AluOpType.max

    with tile.TileContext(nc) as tc:
        with (
            tc.tile_pool(name="const", bufs=1) as cpool,
            tc.tile_pool(name="xin", bufs=6) as xpool,
            tc.tile_pool(name="auxin", bufs=6) as apool,
            tc.tile_pool(name="h1", bufs=4) as h1pool,
            tc.tile_pool(name="h2", bufs=4) as h2pool,
            tc.tile_pool(name="h3", bufs=4) as h3pool,
            tc.tile_pool(name="acc", bufs=1) as accpool,
            tc.tile_pool(name="ps1", bufs=2, space="PSUM") as ps1,
            tc.tile_pool(name="ps2", bufs=2, space="PSUM") as ps2,
            tc.tile_pool(name="ps3", bufs=2, space="PSUM") as ps3,
            tc.tile_pool(name="pso", bufs=1, space="PSUM") as pso,
        ):
            w1s = cpool.tile([C, H], bf16, tag="w1")
            w2s = cpool.tile([H, H], bf16, tag="w2")
            w3s = cpool.tile([H, H], bf16, tag="w3")
            b1s = cpool.tile([H, 1], f32, tag="b1")
            b2s = cpool.tile([H, 1], f32, tag="b2")
            b3s = cpool.tile([H, 1], f32, tag="b3")
            negbig = cpool.tile([1, H], bf16, tag="negbig")
            wsums = cpool.tile([H, O], f32, tag="wsum")
            wmaxs = cpool.tile([H, O], f32, tag="wmax")
            wmeans = cpool.tile([H, O], f32, tag="wmean")
            bos = cpool.tile([1, O], f32, tag="bo")
            recs = cpool.tile([H, S // H], f32, tag="recip")
            ones = cpool.tile([1, H], f32, tag="ones")

            nc.sync.dma_start(w1s[:], w1)
            nc.sync.dma_start(w2s[:], w2)
            nc.sync.dma_start(w3s[:], w3)
            nc.sync.dma_start(b1s[:], b1)
            nc.sync.dma_start(b2s[:], b2)
            nc.sync.dma_start(b3s[:], b3)
            nc.sync.dma_start(wsums[:], wsum)
            nc.sync.dma_start(wmaxs[:], wmax)
            nc.sync.dma_start(wmeans[:], wmean)
            nc.sync.dma_start(bos[:], bo)
            nc.sync.dma_start(recs[:], recip)
            nc.vector.memset(ones[:], 1.0)
            nc.sync.dma_start(negbig[:], nbig)

            # Persistent per-slot partials (post-relu sums and maxes).
            sumP = accpool.tile([H, S], f32, tag="sumP")
            maxP = accpool.tile([H, S], f32, tag="maxP")

            def relu_evac(eng, dst, src, bias):
                if eng == "a":
                    nc.scalar.activation(dst, src, relu, bias=bias)
                else:
                    nc.vector.tensor_scalar(
                        dst, src, bias, 0.0, op0=add, op1=amax
                    )

            for pi in range(0, len(tiles), 2):
                pair = [
                    (ti, tiles[ti])
                    for ti in range(pi, min(pi + 2, len(tiles)))
                ]
                st = {}
                for ti, (k0, d, wt, col0, tailw) in pair:
                    tcols = d * wt
                    xt = xpool.tile([C, MAX_TILE], bf16, tag="xt")
                    at = apool.tile([1, MAX_TILE], bf16, tag="at")
                    nc.sync.dma_start(xt[:, :tcols], xT[:, col0 : col0 + tcols])
                    if tailw > 0:
                        nc.sync.dma_start(
                            at[:, :tcols], aux[:, col0 : col0 + tcols]
                        )
                    st[ti] = dict(xt=xt, at=at, tcols=tcols)

                for ti, (k0, d, wt, col0, tailw) in pair:
                    tcols = st[ti]["tcols"]
                    p1 = ps1.tile([H, MAX_TILE], f32, tag="p1")
                    nc.tensor.matmul(
                        p1[:, :tcols], w1s[:], st[ti]["xt"][:, :tcols]
                    )
                    st[ti]["p1"] = p1
                for ti, (k0, d, wt, col0, tailw) in pair:
                    tcols = st[ti]["tcols"]
                    h1 = h1pool.tile([H, MAX_TILE], bf16, tag="h1")
                    relu_evac(
                        R1_PAT[ti % len(R1_PAT)],
                        h1[:, :tcols], st[ti]["p1"][:, :tcols], b1s[:],
                    )
                    st[ti]["h1"] = h1

                for ti, (k0, d, wt, col0, tailw) in pair:
                    tcols = st[ti]["tcols"]
                    p2 = ps2.tile([H, MAX_TILE], f32, tag="p2")
                    nc.tensor.matmul(
                        p2[:, :tcols], w2s[:], st[ti]["h1"][:, :tcols]
                    )
                    st[ti]["p2"] = p2
                for ti, (k0, d, wt, col0, tailw) in pair:
                    tcols = st[ti]["tcols"]
                    h2 = h2pool.tile([H, MAX_TILE], bf16, tag="h2")
                    relu_evac(
                        R2_PAT[ti % len(R2_PAT)],
                        h2[:, :tcols], st[ti]["p2"][:, :tcols], b2s[:],
                    )
                    st[ti]["h2"] = h2

                for ti, (k0, d, wt, col0, tailw) in pair:
                    tcols = st[ti]["tcols"]
                    p3 = ps3.tile([H, MAX_TILE], f32, tag="p3")
                    nc.tensor.matmul(
                        p3[:, :tcols], w3s[:], st[ti]["h2"][:, :tcols],
                        start=True, stop=(tailw == 0),
                    )
                    st[ti]["p3"] = p3
                for ti, (k0, d, wt, col0, tailw) in pair:
                    if tailw == 0:
                        continue
                    tcols = st[ti]["tcols"]
                    p3v = st[ti]["p3"][:, :tcols].rearrange(
                        "p (d w) -> p d w", d=d
                    )
                    # -BIG into the per-slot tail windows (the only columns
                    # that can be pads on any core); real columns add 0.
                    atv = st[ti]["at"][:, :tcols].rearrange(
                        "p (d w) -> p d w", d=d
                    )
                    nc.tensor.matmul(
                        p3v[:, :, wt - tailw : wt],
                        negbig[:],
                        atv[:, :, wt - tailw : wt],
                        start=False,
                        stop=True,
                    )

                for ti, (k0, d, wt, col0, tailw) in pair:
                    tcols = st[ti]["tcols"]
                    p3 = st[ti]["p3"]
                    h3 = h3pool.tile([H, MAX_TILE], bf16, tag="h3")
                    # Fused relu3 + per-slot segment sum: the Scalar engine's
                    # accumulator sums the post-activation output along the
                    # free axis (one accum_out write per slot window).
                    for j in range(d):
                        wl = slice(j * wt, (j + 1) * wt)
                        nc.scalar.activation(
                            h3[:, wl], p3[:, wl], relu, bias=b3s[:],
                            accum_out=sumP[:, k0 + j : k0 + j + 1],
                        )
                    h3v = h3[:, :tcols].rearrange("p (d w) -> p d w", d=d)
                    nc.vector.reduce_max(
                        maxP[:, k0 : k0 + d], h3v, axis=mybir.AxisListType.X
                    )

            # ---- epilogue: out[k, :] = sum_k @ Wsum + max_k @ Wmax
            #                + (sum_k * recip_k) @ Wmean + bo ----
            for ch in range(S // H):  # 2 chunks of 128 segments
                sl = slice(ch * H, (ch + 1) * H)
                po = pso.tile([H, O], f32, tag="po")
                nc.tensor.matmul(po[:], sumP[:, sl], wsums[:], start=True, stop=False)
                nc.tensor.matmul(po[:], maxP[:, sl], wmaxs[:], start=False, stop=False)
                nc.tensor.matmul(po[:], ones[:], bos[:], start=False, stop=True)

                pm = pso.tile([H, O], f32, tag="pm")
                nc.tensor.matmul(pm[:], sumP[:, sl], wmeans[:], start=True, stop=True)

                om = h1pool.tile([H, O], f32, tag="om")
                nc.vector.tensor_scalar_mul(om[:], pm[:], recs[:, ch : ch + 1])
                ot = h2pool.tile([H, O], f32, tag="ot")
                nc.vector.tensor_tensor(ot[:], po[:], om[:], op=mybir.AluOpType.add)
                nc.sync.dma_start(out[sl, :], ot[:])

    nc.compile()
    return nc


def kernel(**inputs):
    x = np.ascontiguousarray(np.asarray(inputs["x"], dtype=np.float32))
    batch = np.asarray(inputs["batch"]).astype(np.int64)

    # ---- fold BN into the linears ----
    W1p, b1p = _fold_bn(
        np.asarray(inputs["W1"]), np.asarray(inputs["b1"]),
        np.asarray(inputs["g1"]), np.asarray(inputs["be1"]),
        np.asarray(inputs["m1"]), np.asarray(inputs["v1"]),
    )
    W2p, b2p = _fold_bn(
        np.asarray(inputs["W2"]), np.asarray(inputs["b2"]),
        np.asarray(inputs["g2"]), np.asarray(inputs["be2"]),
        np.asarray(inputs["m2"]), np.asarray(inputs["v2"]),
    )
    W3p, b3p = _fold_bn(
        np.asarray(inputs["W3"]), np.asarray(inputs["b3"]),
        np.asarray(inputs["g3"]), np.asarray(inputs["be3"]),
        np.asarray(inputs["m3"]), np.asarray(inputs["v3"]),
    )
    Wop, bop = _fold_bn(
        np.asarray(inputs["Wo"]), np.asarray(inputs["bo"]),
        np.asarray(inputs["go"]), np.asarray(inputs["beo"]),
        np.asarray(inputs["mo"]), np.asarray(inputs["vo"]),
    )

    # Pad columns are zero in x, so h2_pad is a known constant; BIG pushes the
    # padded layer-3 pre-activation strictly below zero.  Round BIG to a
    # bf16-exact value.
    h1_pad = np.maximum(b1p, 0.0)
    h2_pad = np.maximum(W2p @ h1_pad + b2p, 0.0)
    v3 = W3p @ h2_pad + b3p
    BIG = float(np.float32(BF16(max(0.0, float(v3.max())) + 1024.0)))

    # ---- whole-segment sharding by sorted-width round-robin rank ----
    counts = np.bincount(batch, minlength=NSEG).astype(np.int64)
    assert np.all(batch[:-1] <= batch[1:]), "batch must be sorted"
    order = np.argsort(-counts, kind="stable")  # segment ids, width desc
    slot_w = np.maximum(counts[order[::NCORES][:S]], 1)  # width of rank 8k
    tiles0, cols = _plan_tiles(slot_w)

    # per-tile tail-window width: the trailing columns of each slot that can
    # be padding on ANY core (only these need the -BIG mask matmul)
    wmat = counts[order[: S * NCORES]].reshape(S, NCORES)  # slot x core widths
    tiles = []
    for k0, d, wt, col0 in tiles0:
        minw = int(wmat[k0 : k0 + d].min())
        tailw = min(wt, (wt - minw + 1) & ~1)
        tiles.append((k0, d, wt, col0, tailw))

    key = (cols, float(BIG), tuple(slot_w.tolist()),
           tuple(t[4] for t in tiles))
    if key not in _compiled_cache:
        _compiled_cache[key] = _build_program(tiles, cols, BIG)
    nc = _compiled_cache[key]

    # column start of each slot
    slot_col = np.zeros(S, dtype=np.int64)
    for k0, d, wt, col0, tailw in tiles:
        for j in range(d):
            slot_col[k0 + j] = col0 + j * wt

    starts = np.searchsorted(batch, np.arange(NSEG), side="left")
    ends = np.searchsorted(batch, np.arange(NSEG), side="right")

    in_maps = []
    for c in range(NCORES):
        segs = order[np.arange(S) * NCORES + c]  # this core's segment ids
        src = np.full(cols, -1, dtype=np.int64)
        for k in range(S):
            s = segs[k]
            cnt = int(counts[s])
            if cnt:
                src[slot_col[k] : slot_col[k] + cnt] = np.arange(
                    starts[s], ends[s]
                )
        real = src >= 0
        xTc = np.zeros((C, cols), dtype=BF16)
        xTc[:, real] = x[src[real]].astype(BF16).T
        auxc = np.zeros((1, cols), dtype=BF16)
        auxc[0, ~real] = 1.0
        recipc = (1.0 / np.maximum(counts[segs], 1.0)).astype(np.float32)
        in_maps.append(
            dict(
                xT=xTc,
                aux=auxc,
                w1=np.ascontiguousarray(W1p.T.astype(BF16)),
                w2=np.ascontiguousarray(W2p.T.astype(BF16)),
                w3=np.ascontiguousarray(W3p.T.astype(BF16)),
                b1=np.ascontiguousarray(b1p[:, None]),
                b2=np.ascontiguousarray(b2p[:, None]),
                b3=np.ascontiguousarray(b3p[:, None]),
                nbig=np.full((1, H), -BIG, BF16),
                wsum=np.ascontiguousarray(Wop[:, 0:H].T),
                wmax=np.ascontiguousarray(Wop[:, H : 2 * H].T),
                wmean=np.ascontiguousarray(Wop[:, 2 * H : 3 * H].T),
                bo=np.ascontiguousarray(bop[None, :]),
                recip=np.ascontiguousarray(recipc.reshape(S // H, H).T),
            )
        )

    ncores_run = int(os.environ.get("KERNEL_NCORES", str(NCORES)))
    res = bass_utils.run_bass_kernel_spmd(
        nc,
        in_maps[:ncores_run],
        core_ids=list(range(ncores_run)),
        trace=bool(int(os.environ.get("KERNEL_TRACE", "0"))),
        tmpdir=os.environ.get("KERNEL_TRACE_DIR") or None,
    )
    kernel.last_results = res

    out_full = np.zeros((NSEG, O), dtype=np.float32)
    ranks = np.arange(S)
    for c in range(ncores_run):
        out_full[order[ranks * NCORES + c]] = res.results[c]["out"]
    return out_full


# revision 23
# speedup vs baseline: 3.8204x; 1.0140x over previous
"""DeepSets segment-reduce kernel for 8x Trainium2 NeuronCores.

Strategy (all shapes hardcoded for N=500000, C=H=128, O=64, NSEG=2048):
  - Transposed activation layout: features on SBUF partitions, nodes on the
    free axis, so segment reductions are free-axis reduces.
  - Whole-segment sharding: every segment is assigned entirely to one core,
    round-robin by global sorted-width rank.  All 8 cores then share an
    identical compile-time slot/tile geometry (SPMD-safe); per-core padding
    is <1%.  No collective is needed - the host gather is the unshard.
  - Encoder BN is folded into the linear weights (W' = W * g*rsqrt(v+eps),
    b' = (b-m)*g*rsqrt(v+eps) + beta), so each layer is relu(W'x + b').
  - All encoder matmul operands are bf16: the PE streams bf16 moving rows
    at 1 cycle/row vs ~2 for fp32/f32r, and the x DMA halves.  PSUM
    accumulation stays fp32; rel-err vs the fp32 reference is ~2e-3.
  - A large negative pad mask is injected into layer-3 PSUM by a rank-1
    matmul (-BIG x is_pad) over each slot's tail window.  Pad columns then
    fall below zero pre-relu, so they contribute exactly 0 to the post-relu
    segment sums and maxes.
  - Engine balance: layer 3 is evacuated by per-slot Scalar-engine
    activation ops whose accum_out accumulates the slot's post-relu segment
    sum for free; relu2 runs on the Vector engine and relu1 is split between
    the two by a per-tile pattern.  The segment max is a whole-tile 3D
    reduce_max on DVE over the bf16 h3 (post-relu, bias applied, pads zero -
    so maxP needs no epilogue fixup).
  - Tiles are processed in pairs with matmuls grouped per layer, so each
    PE weight load (LdWeights) covers two 512-column matmuls.
  - Final projection out = [sum|max|mean] @ Wo'.T + bo' runs per core on its
    own 256 segments; mean is handled by projecting sums through the mean
    block of Wo' and row-scaling by 1/count.
"""

import os
import sys

import numpy as np

if "/opt/trn_rl_repo" not in sys.path:
    sys.path.insert(0, "/opt/trn_rl_repo")

import ml_dtypes

import concourse.bacc as bacc
import concourse.mybir as mybir
import concourse.tile as tile
from concourse import bass_utils

EPS = 1e-5
NSEG = 2048
NCORES = 8
C = 128
H = 128
O = 64
S = NSEG // NCORES  # segment slots per core (256)
MAX_TILE = 512  # PSUM bank / moving-operand limit

BF16 = ml_dtypes.bfloat16

# Per-tile engine assignment patterns for the relu1/relu2 evacuations
# ("a" = Scalar/ACT, "d" = Vector/DVE).  Tuned from trace engine-busy%.
R1_PAT = "aad"
R2_PAT = "d"

_compiled_cache = {}


def _fold_bn(W, b, g, be, m, v):
    a = g / np.sqrt(v + EPS)
    Wp = W * a[:, None]
    bp = (b - m) * a + be
    return Wp.astype(np.float32), bp.astype(np.float32)


def _plan_tiles(slot_w):
    """Greedy-pack slots (widths descending) into tiles of <=MAX_TILE cols.

    Returns list of (slot_start, n_slots, padded_width, col_start) and the
    total padded column count.
    """
    tiles = []
    col = 0
    k = 0
    n = len(slot_w)
    while k < n:
        wt = (int(slot_w[k]) + 1) & ~1  # keep matmul widths even
        assert 0 < wt <= MAX_TILE, f"slot width {wt} unsupported"
        d = min(MAX_TILE // wt, n - k)
        tiles.append((k, d, wt, col))
        col += d * wt
        k += d
    return tiles, col


def _build_program(tiles, cols, BIG_DEVICE):
    """Emit the Bass/Tile program shared by all 8 cores."""
    nc = bacc.Bacc(
        "TRN2",
        target_bir_lowering=False,
        debug=False,
        num_devices=NCORES,
    )
    f32 = mybir.dt.float32
    bf16 = mybir.dt.bfloat16

    xT = nc.dram_tensor("xT", [C, cols], bf16, kind="ExternalInput").ap()
    aux = nc.dram_tensor("aux", [1, cols], bf16, kind="ExternalInput").ap()
    w1 = nc.dram_tensor("w1", [C, H], bf16, kind="ExternalInput").ap()
    w2 = nc.dram_tensor("w2", [H, H], bf16, kind="ExternalInput").ap()
    w3 = nc.dram_tensor("w3", [H, H], bf16, kind="ExternalInput").ap()
    b1 = nc.dram_tensor("b1", [H, 1], f32, kind="ExternalInput").ap()
    b2 = nc.dram_tensor("b2", [H, 1], f32, kind="ExternalInput").ap()
    b3 = nc.dram_tensor("b3", [H, 1], f32, kind="ExternalInput").ap()
    nbig = nc.dram_tensor("nbig", [1, H], bf16, kind="ExternalInput").ap()
    wsum = nc.dram_tensor("wsum", [H, O], f32, kind="ExternalInput").ap()
    wmax = nc.dram_tensor("wmax", [H, O], f32, kind="ExternalInput").ap()
    wmean = nc.dram_tensor("wmean", [H, O], f32, kind="ExternalInput").ap()
    bo = nc.dram_tensor("bo", [1, O], f32, kind="ExternalInput").ap()
    # column ch holds the reciprocals for segment chunk ch (128 slots each)
    recip = nc.dram_tensor("recip", [H, S // H], f32, kind="ExternalInput").ap()
    out = nc.dram_tensor("out", [S, O], f32, kind="ExternalOutput").ap()

    relu = mybir.ActivationFunctionType.Relu
    add = mybir.AluOpType.add
    amax = mybir.AluOpType.max

    with tile.TileContext(nc) as tc:
        with (
            tc.tile_pool(name="const", bufs=1) as cpool,
            tc.tile_pool(name="xin", bufs=8) as xpool,
            tc.tile_pool(name="auxin", bufs=8) as apool,
            tc.tile_pool(name="h1", bufs=6) as h1pool,
            tc.tile_pool(name="h2", bufs=6) as h2pool,
            tc.tile_pool(name="h3", bufs=6) as h3pool,
            tc.tile_pool(name="acc", bufs=1) as accpool,
            tc.tile_pool(name="ps1", bufs=3, space="PSUM") as ps1,
            tc.tile_pool(name="ps2", bufs=3, space="PSUM") as ps2,
            tc.tile_pool(name="ps3", bufs=2, space="PSUM") as ps3,
        ):
            w1s = cpool.tile([C, H], bf16, tag="w1")
            w2s = cpool.tile([H, H], bf16, tag="w2")
            w3s = cpool.tile([H, H], bf16, tag="w3")
            b1s = cpool.tile([H, 1], f32, tag="b1")
            b2s = cpool.tile([H, 1], f32, tag="b2")
            b3s = cpool.tile([H, 1], f32, tag="b3")
            negbig = cpool.tile([1, H], bf16, tag="negbig")
            wsums = cpool.tile([H, O], f32, tag="wsum")
            wmaxs = cpool.tile([H, O], f32, tag="wmax")
            wmeans = cpool.tile([H, O], f32, tag="wmean")
            bos = cpool.tile([1, O], f32, tag="bo")
            recs = cpool.tile([H, S // H], f32, tag="recip")
            ones = cpool.tile([1, H], f32, tag="ones")

            nc.sync.dma_start(w1s[:], w1)
            nc.sync.dma_start(w2s[:], w2)
            nc.sync.dma_start(w3s[:], w3)
            nc.sync.dma_start(b1s[:], b1)
            nc.sync.dma_start(b2s[:], b2)
            nc.sync.dma_start(b3s[:], b3)
            nc.sync.dma_start(wsums[:], wsum)
            nc.sync.dma_start(wmaxs[:], wmax)
            nc.sync.dma_start(wmeans[:], wmean)
            nc.sync.dma_start(bos[:], bo)
            nc.sync.dma_start(recs[:], recip)
            nc.vector.memset(ones[:], 1.0)
            nc.sync.dma_start(negbig[:], nbig)

            # Persistent per-slot partials (post-relu sums and maxes).
            sumP = accpool.tile([H, S], f32, tag="sumP")
            maxP = accpool.tile([H, S], f32, tag="maxP")

            def relu_evac(eng, dst, src, bias):
                if eng == "a":
                    nc.scalar.activation(dst, src, relu, bias=bias)
                else:
                    nc.vector.tensor_scalar(
                        dst, src, bias, 0.0, op0=add, op1=amax
                    )

            for pi in range(0, len(tiles), 2):
                pair = [
                    (ti, tiles[ti])
                    for ti in range(pi, min(pi + 2, len(tiles)))
                ]
                st = {}
                for ti, (k0, d, wt, col0, tailw) in pair:
                    tcols = d * wt
                    xt = xpool.tile([C, MAX_TILE], bf16, tag="xt")
                    at = apool.tile([1, MAX_TILE], bf16, tag="at")
                    nc.sync.dma_start(xt[:, :tcols], xT[:, col0 : col0 + tcols])
                    if tailw > 0:
                        nc.sync.dma_start(
                            at[:, :tcols], aux[:, col0 : col0 + tcols]
                        )
                    st[ti] = dict(xt=xt, at=at, tcols=tcols)

                for ti, (k0, d, wt, col0, tailw) in pair:
                    tcols = st[ti]["tcols"]
                    p1 = ps1.tile([H, MAX_TILE], f32, tag="p1")
                    nc.tensor.matmul(
                        p1[:, :tcols], w1s[:], st[ti]["xt"][:, :tcols]
                    )
                    st[ti]["p1"] = p1
                for ti, (k0, d, wt, col0, tailw) in pair:
                    tcols = st[ti]["tcols"]
                    h1 = h1pool.tile([H, MAX_TILE], bf16, tag="h1")
                    relu_evac(
                        R1_PAT[ti % len(R1_PAT)],
                        h1[:, :tcols], st[ti]["p1"][:, :tcols], b1s[:],
                    )
                    st[ti]["h1"] = h1

                for ti, (k0, d, wt, col0, tailw) in pair:
                    tcols = st[ti]["tcols"]
                    p2 = ps2.tile([H, MAX_TILE], f32, tag="p2")
                    nc.tensor.matmul(
                        p2[:, :tcols], w2s[:], st[ti]["h1"][:, :tcols]
                    )
                    st[ti]["p2"] = p2
                for ti, (k0, d, wt, col0, tailw) in pair:
                    tcols = st[ti]["tcols"]
                    h2 = h2pool.tile([H, MAX_TILE], bf16, tag="h2")
                    relu_evac(
                        R2_PAT[ti % len(R2_PAT)],
                        h2[:, :tcols], st[ti]["p2"][:, :tcols], b2s[:],
                    )
                    st[ti]["h2"] = h2

                for ti, (k0, d, wt, col0, tailw) in pair:
                    tcols = st[ti]["tcols"]
                    p3 = ps3.tile([H, MAX_TILE], f32, tag="p3")
                    nc.tensor.matmul(
                        p3[:, :tcols], w3s[:], st[ti]["h2"][:, :tcols],
                        start=True, stop=(tailw == 0),
                    )
                    st[ti]["p3"] = p3
                for ti, (k0, d, wt, col0, tailw) in pair:
                    if tailw == 0:
                        continue
                    tcols = st[ti]["tcols"]
                    p3v = st[ti]["p3"][:, :tcols].rearrange(
                        "p (d w) -> p d w", d=d
                    )
                    # -BIG into the per-slot tail windows (the only columns
                    # that can be pads on any core); real columns add 0.
                    atv = st[ti]["at"][:, :tcols].rearrange(
                        "p (d w) -> p d w", d=d
                    )
                    nc.tensor.matmul(
                        p3v[:, :, wt - tailw : wt],
                        negbig[:],
                        atv[:, :, wt - tailw : wt],
                        start=False,
                        stop=True,
                    )

                for ti, (k0, d, wt, col0, tailw) in pair:
                    tcols = st[ti]["tcols"]
                    p3 = st[ti]["p3"]
                    h3 = h3pool.tile([H, MAX_TILE], bf16, tag="h3")
                    # Fused relu3 + per-slot segment sum: the Scalar engine's
                    # accumulator sums the post-activation output along the
                    # free axis (one accum_out write per slot window).
                    for j in range(d):
                        wl = slice(j * wt, (j + 1) * wt)
                        nc.scalar.activation(
                            h3[:, wl], p3[:, wl], relu, bias=b3s[:],
                            accum_out=sumP[:, k0 + j : k0 + j + 1],
                        )
                    h3v = h3[:, :tcols].rearrange("p (d w) -> p d w", d=d)
                    nc.vector.reduce_max(
                        maxP[:, k0 : k0 + d], h3v, axis=mybir.AxisListType.X
                    )

            # ---- epilogue: out[k, :] = sum_k @ Wsum + max_k @ Wmax
            #                + (sum_k * recip_k) @ Wmean + bo ----
            for ch in range(S // H):  # 2 chunks of 128 segments
                sl = slice(ch * H, (ch + 1) * H)
                pot = ps3.tile([H, MAX_TILE], f32, tag="p3")
                po = pot[:, :O]
                nc.tensor.matmul(po, sumP[:, sl], wsums[:], start=True, stop=False)
                nc.tensor.matmul(po, maxP[:, sl], wmaxs[:], start=False, stop=False)
                nc.tensor.matmul(po, ones[:], bos[:], start=False, stop=True)

                pmt = ps3.tile([H, MAX_TILE], f32, tag="p3")
                pm = pmt[:, :O]
                nc.tensor.matmul(pm, sumP[:, sl], wmeans[:], start=True, stop=True)

                om = h1pool.tile([H, O], f32, tag="om")
                nc.vector.tensor_scalar_mul(om[:], pm, recs[:, ch : ch + 1])
                ot = h2pool.tile([H, O], f32, tag="ot")
                nc.vector.tensor_tensor(ot[:], po, om[:], op=mybir.AluOpType.add)
                nc.sync.dma_start(out[sl, :], ot[:])

    nc.compile()
    return nc


def kernel(**inputs):
    x = np.ascontiguousarray(np.asarray(inputs["x"], dtype=np.float32))
    batch = np.asarray(inputs["batch"]).astype(np.int64)

    # ---- fold BN into the linears ----
    W1p, b1p = _fold_bn(
        np.asarray(inputs["W1"]), np.asarray(inputs["b1"]),
        np.asarray(inputs["g1"]), np.asarray(inputs["be1"]),
        np.asarray(inputs["m1"]), np.asarray(inputs["v1"]),
    )
    W2p, b2p = _fold_bn(
        np.asarray(inputs["W2"]), np.asarray(inputs["b2"]),
        np.asarray(inputs["g2"]), np.asarray(inputs["be2"]),
        np.asarray(inputs["m2"]), np.asarray(inputs["v2"]),
    )
    W3p, b3p = _fold_bn(
        np.asarray(inputs["W3"]), np.asarray(inputs["b3"]),
        np.asarray(inputs["g3"]), np.asarray(inputs["be3"]),
        np.asarray(inputs["m3"]), np.asarray(inputs["v3"]),
    )
    Wop, bop = _fold_bn(
        np.asarray(inputs["Wo"]), np.asarray(inputs["bo"]),
        np.asarray(inputs["go"]), np.asarray(inputs["beo"]),
        np.asarray(inputs["mo"]), np.asarray(inputs["vo"]),
    )

    # Pad columns are zero in x, so h2_pad is a known constant; BIG pushes the
    # padded layer-3 pre-activation strictly below zero.  Round BIG to a
    # bf16-exact value.
    h1_pad = np.maximum(b1p, 0.0)
    h2_pad = np.maximum(W2p @ h1_pad + b2p, 0.0)
    v3 = W3p @ h2_pad + b3p
    BIG = float(np.float32(BF16(max(0.0, float(v3.max())) + 1024.0)))

    # ---- whole-segment sharding by sorted-width round-robin rank ----
    counts = np.bincount(batch, minlength=NSEG).astype(np.int64)
    assert np.all(batch[:-1] <= batch[1:]), "batch must be sorted"
    order = np.argsort(-counts, kind="stable")  # segment ids, width desc
    slot_w = np.maximum(counts[order[::NCORES][:S]], 1)  # width of rank 8k
    tiles0, cols = _plan_tiles(slot_w)

    # per-tile tail-window width: the trailing columns of each slot that can
    # be padding on ANY core (only these need the -BIG mask matmul)
    wmat = counts[order[: S * NCORES]].reshape(S, NCORES)  # slot x core widths
    tiles = []
    for k0, d, wt, col0 in tiles0:
        minw = int(wmat[k0 : k0 + d].min())
        tailw = min(wt, (wt - minw + 1) & ~1)
        tiles.append((k0, d, wt, col0, tailw))

    key = (cols, float(BIG), tuple(slot_w.tolist()),
           tuple(t[4] for t in tiles))
    if key not in _compiled_cache:
        _compiled_cache[key] = _build_program(tiles, cols, BIG)
    nc = _compiled_cache[key]

    # column start of each slot
    slot_col = np.zeros(S, dtype=np.int64)
    for k0, d, wt, col0, tailw in tiles:
        for j in range(d):
            slot_col[k0 + j] = col0 + j * wt

    starts = np.searchsorted(batch, np.arange(NSEG), side="left")
    ends = np.searchsorted(batch, np.arange(NSEG), side="right")

    in_maps = []
    for c in range(NCORES):
        segs = order[np.arange(S) * NCORES + c]  # this core's segment ids
        src = np.full(cols, -1, dtype=np.int64)
        for k in range(S):
            s = segs[k]
            cnt = int(counts[s])
            if cnt:
                src[slot_col[k] : slot_col[k] + cnt] = np.arange(
                    starts[s], ends[s]
                )
        real = src >= 0
        xTc = np.zeros((C, cols), dtype=BF16)
        xTc[:, real] = x[src[real]].astype(BF16).T
        auxc = np.zeros((1, cols), dtype=BF16)
        auxc[0, ~real] = 1.0
        recipc = (1.0 / np.maximum(counts[segs], 1.0)).astype(np.float32)
        in_maps.append(
            dict(
                xT=xTc,
                aux=auxc,
                w1=np.ascontiguousarray(W1p.T.astype(BF16)),
                w2=np.ascontiguousarray(W2p.T.astype(BF16)),
                w3=np.ascontiguousarray(W3p.T.astype(BF16)),
                b1=np.ascontiguousarray(b1p[:, None]),
                b2=np.ascontiguousarray(b2p[:, None]),
                b3=np.ascontiguousarray(b3p[:, None]),
                nbig=np.full((1, H), -BIG, BF16),
                wsum=np.ascontiguousarray(Wop[:, 0:H].T),
                wmax=np.ascontiguousarray(Wop[:, H : 2 * H].T),
                wmean=np.ascontiguousarray(Wop[:, 2 * H : 3 * H].T),
                bo=np.ascontiguousarray(bop[None, :]),
                recip=np.ascontiguousarray(recipc.reshape(S // H, H).T),
            )
        )

    ncores_run = int(os.environ.get("KERNEL_NCORES", str(NCORES)))
    res = bass_utils.run_bass_kernel_spmd(
        nc,
        in_maps[:ncores_run],
        core_ids=list(range(ncores_run)),
        trace=bool(int(os.environ.get("KERNEL_TRACE", "0"))),
        tmpdir=os.environ.get("KERNEL_TRACE_DIR") or None,
    )
    kernel.last_results = res

    out_full = np.zeros((NSEG, O), dtype=np.float32)
    ranks = np.arange(S)
    for c in range(ncores_run):
        out_full[order[ranks * NCORES + c]] = res.results[c]["out"]
    return out_full
